# revision 6
# baseline (speedup 1.0000x reference)
"""BandSplit Trainium2 kernel.

out[b, d, t, k] = sum_{c,w} x[b, c, t, idx[k, w]] * pre_w[k, c*W + w, d] + pre_b[k, d]

Bands are contiguous frequency ranges (mel triangles), so the ragged
gather is a banded sparse matmul. Strategy (8 cores, shard T 8-way):

  * Each core takes t in [core*128, core*128+128) for all 4 batches
    (512 (b,t) rows).
  * x is loaded as (t=128 partitions, 33 chunks x [c*32 + w] cols) --
    the load DMA interleaves the 4 channels at 32-frequency-column
    granularity. PE transpose of each 128x128 block then yields
    xT[chunk] = (4c*32f = 128 partitions, 4b*128t = 512 free): the
    matmul contraction layout.
  * Weights are host-prepacked per (band, 32-aligned f-chunk) incidence
    into 128x128 fp32 tiles (zero rows outside the band) whose row
    order matches xT partitions. Per band: accumulate its incidences
    into one PSUM (d=128, 512) tile, fused bias-add on ScalarE into a
    grouped stage tile, DMA groups of 8 bands to a device-friendly
    (K, D, B*128) output. Host reassembles.
"""

import numpy as np

import concourse.bass as bass
import concourse.tile as tile
from concourse import bacc, mybir
from concourse.bass_utils import run_bass_kernel_spmd

F32 = mybir.dt.float32

N_CORES = 8
G = 32  # channel-interleave granularity (f columns per c segment)
WGRP = 32  # weight chunks per mega-DMA
KGRP = 8  # bands per output stage tile / DMA

_cache = {}


def _band_ranges(indices_pad, mask):
    W = mask.sum(axis=1).astype(np.int64)
    starts = indices_pad[:, 0].astype(np.int64)
    ends = starts + W
    return starts, ends


def _pack_weights(pre_w, starts, ends, Wmax):
    """(NINC, 128, 128) fp32 tiles; rows p = c*G + (f - 32*j) matching xT."""
    K, CW, D = pre_w.shape
    C = CW // Wmax
    pw = pre_w.reshape(K, C, Wmax, D)
    incid = []  # (k, j) pairs in band-major order
    for k in range(K):
        for j in range(int(starts[k]) // G, (int(ends[k]) - 1) // G + 1):
            incid.append((k, j))
    wchunks = np.zeros((len(incid), 128, D), np.float32)
    for i, (k, j) in enumerate(incid):
        f0 = max(int(starts[k]), G * j)
        f1 = min(int(ends[k]), G * (j + 1))
        w0 = f0 - int(starts[k])
        p0 = f0 - G * j
        n = f1 - f0
        for c in range(C):
            wchunks[i, c * G + p0 : c * G + p0 + n, :] = pw[k, c, w0 : w0 + n, :]
    return wchunks, incid


def _build_program(Tc, F, n_chunks, band_chunks, n_inc):
    """Bass program for one core. Tc = timesteps per core (128)."""
    B, C, K, D = 4, 4, 64, 128
    NF = n_chunks * 128  # xnat free size
    BT = B * Tc
    n_wgrp = (n_inc + WGRP - 1) // WGRP

    nc = bacc.Bacc("TRN2", target_bir_lowering=False, debug=False)
    x_ap = nc.dram_tensor("x", [B, C, Tc, F], F32, kind="ExternalInput").ap()
    w_ap = nc.dram_tensor("w", [n_wgrp * WGRP, 128, D], F32, kind="ExternalInput").ap()
    bias_ap = nc.dram_tensor("bias", [D, K], F32, kind="ExternalInput").ap()
    ident_ap = nc.dram_tensor("ident", [128, 128], F32, kind="ExternalInput").ap()
    out_ap = nc.dram_tensor("out", [K, D, BT], F32, kind="ExternalOutput").ap()

    # DMA-issuing engines for the x interleave loads, round-robin
    x_engines = [nc.sync, nc.scalar, nc.gpsimd, nc.sync]

    nfull = F // G  # full 32-col f chunks (32)
    with tile.TileContext(nc) as tc:
        from contextlib import ExitStack

        with ExitStack() as ctx:
            const_pool = ctx.enter_context(tc.tile_pool(name="const", bufs=1))
            xnat_pool = ctx.enter_context(tc.tile_pool(name="xnat", bufs=2))
            xt_pool = ctx.enter_context(tc.tile_pool(name="xt", bufs=1))
            w_pool = ctx.enter_context(tc.tile_pool(name="w", bufs=3))
            st_pool = ctx.enter_context(tc.tile_pool(name="st", bufs=2))
            pt_pool = ctx.enter_context(tc.tile_pool(name="pt", bufs=3, space="PSUM"))
            pm_pool = ctx.enter_context(tc.tile_pool(name="pm", bufs=4, space="PSUM"))

            ident = const_pool.tile([128, 128], F32, tag="ident")
            nc.sync.dma_start(ident[:], ident_ap[:])
            biasT = const_pool.tile([D, K], F32, tag="bias")
            nc.sync.dma_start(biasT[:], bias_ap[:])

            xT = [
                xt_pool.tile([128, BT], F32, tag=f"xT{j}", name=f"xT{j}")
                for j in range(n_chunks)
            ]

            # weight mega-loads: groups of WGRP chunks, (128, WGRP*128) tiles
            w3 = w_ap.rearrange("(g n) p d -> g p n d", n=WGRP)
            wtiles = []
            for g in range(n_wgrp):
                wt = w_pool.tile(
                    [128, WGRP, D], F32, tag="wg", name=f"wg{g}"
                )
                nc.sync.dma_start(wt[:], w3[g])
                wtiles.append(wt)

            for b in range(B):
                xn = xnat_pool.tile([Tc, NF], F32, tag="xnat", name=f"xn{b}")
                if nfull < n_chunks:
                    nc.gpsimd.memset(xn[:, nfull * 128 : n_chunks * 128], 0.0)
                xn3 = xn.rearrange("p (j z) -> p j z", z=128)
                for c in range(C):
                    src = x_ap[b, c][:, : nfull * G].rearrange(
                        "p (j w) -> p j w", w=G
                    )
                    x_engines[c].dma_start(xn3[:, :nfull, c * G : (c + 1) * G], src)
                    for fi in range(nfull * G, F):
                        j = fi // G
                        w = fi - G * j
                        nc.gpsimd.dma_start(
                            xn3[:, j, c * G + w : c * G + w + 1],
                            x_ap[b, c][:, fi : fi + 1],
                        )
                for j in range(n_chunks):
                    pt = pt_pool.tile([128, Tc], F32, tag="pt")
                    nc.tensor.transpose(pt[:], xn3[:, j], ident[:])
                    if j % 2 == 0:
                        nc.vector.tensor_copy(
                            out=xT[j][:, b * Tc : (b + 1) * Tc], in_=pt[:]
                        )
                    else:
                        nc.scalar.copy(
                            out=xT[j][:, b * Tc : (b + 1) * Tc], in_=pt[:]
                        )

            inc = 0
            st = None
            for k in range(K):
                chunks = band_chunks[k]
                pm = pm_pool.tile([D, BT], F32, tag="pm")
                for idx, j in enumerate(chunks):
                    nc.tensor.matmul(
                        pm[:],
                        wtiles[inc // WGRP][:, inc % WGRP],
                        xT[j][:],
                        start=(idx == 0),
                        stop=(idx == len(chunks) - 1),
                    )
                    inc += 1
                if k % KGRP == 0:
                    st = st_pool.tile([D, KGRP * BT], F32, tag="st")
                nc.scalar.activation(
                    st[:, (k % KGRP) * BT : (k % KGRP + 1) * BT],
                    pm[:],
                    mybir.ActivationFunctionType.Identity,
                    bias=biasT[:, k : k + 1],
                )
                if k % KGRP == KGRP - 1:
                    k0 = k - (KGRP - 1)
                    dst = out_ap[k0 : k0 + KGRP].rearrange("k d t -> d k t")
                    nc.gpsimd.dma_start(
                        dst, st.rearrange("d (k t) -> d k t", k=KGRP)
                    )

    nc.compile()
    return nc


def kernel(x, pre_w, pre_b, indices_pad, mask):
    x = np.asarray(x, np.float32)
    pre_w = np.asarray(pre_w, np.float32)
    pre_b = np.asarray(pre_b, np.float32)
    indices_pad = np.asarray(indices_pad)
    mask_np = np.asarray(mask)

    B, C, T, F = x.shape
    K, CW, D = pre_w.shape
    starts, ends = _band_ranges(indices_pad, mask_np)
    Wmax = CW // C
    n_chunks = (F + G - 1) // G  # 33
    assert T % N_CORES == 0
    Tc = T // N_CORES

    wchunks, incid = _pack_weights(pre_w, starts, ends, Wmax)
    n_inc = len(incid)
    band_chunks = [[] for _ in range(K)]
    for k, j in incid:
        band_chunks[k].append(j)
    # pad weight array to a multiple of WGRP chunks
    n_wgrp = (n_inc + WGRP - 1) // WGRP
    if n_wgrp * WGRP > n_inc:
        pad = np.zeros((n_wgrp * WGRP - n_inc, 128, D), np.float32)
        wchunks = np.concatenate([wchunks, pad], axis=0)

    key = (B, C, T, F, K, D, tuple(starts.tolist()), tuple(ends.tolist()))
    if key not in _cache:
        _cache[key] = _build_program(Tc, F, n_chunks, band_chunks, n_inc)
    nc = _cache[key]

    biasT = np.ascontiguousarray(pre_b.T)
    ident = np.eye(128, dtype=np.float32)
    in_maps = []
    for core in range(N_CORES):
        xs = np.ascontiguousarray(x[:, :, core * Tc : (core + 1) * Tc, :])
        in_maps.append({"x": xs, "w": wchunks, "bias": biasT, "ident": ident})

    global _last_in_maps
    _last_in_maps = in_maps
    res = run_bass_kernel_spmd(nc, in_maps, list(range(N_CORES)))

    # per-core out: (K, D, B*Tc) -> full (B, D, T, K)
    arr = np.stack([res.results[i]["out"] for i in range(N_CORES)])
    arr = arr.reshape(N_CORES, K, D, B, Tc)
    out = np.transpose(arr, (3, 2, 0, 4, 1)).reshape(B, D, T, K)
    return np.ascontiguousarray(out)


# revision 9
# speedup vs baseline: 1.0367x; 1.0367x over previous
"""BandSplit Trainium2 kernel.

out[b, d, t, k] = sum_{c,w} x[b, c, t, idx[k, w]] * pre_w[k, c*W + w, d] + pre_b[k, d]

Bands are contiguous frequency ranges (mel triangles), so the ragged
gather is a banded sparse matmul. Strategy (8 cores, shard T 8-way):

  * Each core takes t in [core*128, core*128+128) for all 4 batches
    (512 (b,t) rows).
  * x is loaded as (t=128 partitions, 33 chunks x [c*32 + w] cols) --
    the load DMA interleaves the 4 channels at 32-frequency-column
    granularity. PE transpose of each 128x128 block then yields
    xT[chunk] = (4c*32f = 128 partitions, 4b*128t = 512 free): the
    matmul contraction layout.
  * Weights are host-prepacked per (band, 32-aligned f-chunk) incidence
    into 128x128 fp32 tiles (zero rows outside the band) whose row
    order matches xT partitions. Per band: accumulate its incidences
    into one PSUM (d=128, 512) tile, fused bias-add on ScalarE into a
    grouped stage tile, DMA groups of 8 bands to a device-friendly
    (K, D, B*128) output. Host reassembles.
"""

import numpy as np

import concourse.bass as bass
import concourse.tile as tile
from concourse import bacc, mybir
from concourse.bass_utils import run_bass_kernel_spmd

F32 = mybir.dt.float32

N_CORES = 8
G = 32  # channel-interleave granularity (f columns per c segment)
WGRP = 32  # weight chunks per mega-DMA
KGRP = 8  # bands per output stage tile / DMA

_cache = {}


def _band_ranges(indices_pad, mask):
    W = mask.sum(axis=1).astype(np.int64)
    starts = indices_pad[:, 0].astype(np.int64)
    ends = starts + W
    return starts, ends


def _pack_weights(pre_w, starts, ends, Wmax):
    """(NINC, 128, 128) fp32 tiles; rows p = c*G + (f - 32*j) matching xT."""
    K, CW, D = pre_w.shape
    C = CW // Wmax
    pw = pre_w.reshape(K, C, Wmax, D)
    incid = []  # (k, j) pairs in band-major order
    for k in range(K):
        for j in range(int(starts[k]) // G, (int(ends[k]) - 1) // G + 1):
            incid.append((k, j))
    wchunks = np.zeros((len(incid), 128, D), np.float32)
    for i, (k, j) in enumerate(incid):
        f0 = max(int(starts[k]), G * j)
        f1 = min(int(ends[k]), G * (j + 1))
        w0 = f0 - int(starts[k])
        p0 = f0 - G * j
        n = f1 - f0
        for c in range(C):
            wchunks[i, c * G + p0 : c * G + p0 + n, :] = pw[k, c, w0 : w0 + n, :]
    return wchunks, incid


def _build_program(Tc, F, n_chunks, band_chunks, n_inc):
    """Bass program for one core. Tc = timesteps per core (128)."""
    B, C, K, D = 4, 4, 64, 128
    NF = n_chunks * 128  # xnat free size
    BT = B * Tc
    n_wgrp = (n_inc + WGRP - 1) // WGRP

    nc = bacc.Bacc("TRN2", target_bir_lowering=False, debug=False)
    x_ap = nc.dram_tensor("x", [B, C, Tc, F], F32, kind="ExternalInput").ap()
    w_ap = nc.dram_tensor("w", [n_wgrp * WGRP, 128, D], F32, kind="ExternalInput").ap()
    bias_ap = nc.dram_tensor("bias", [D, K], F32, kind="ExternalInput").ap()
    ident_ap = nc.dram_tensor("ident", [128, 128], F32, kind="ExternalInput").ap()
    out_ap = nc.dram_tensor("out", [K, D, BT], F32, kind="ExternalOutput").ap()

    # DMA-issuing engines for the x interleave loads, round-robin
    x_engines = [nc.gpsimd, nc.scalar, nc.gpsimd, nc.scalar]

    nfull = F // G  # full 32-col f chunks (32)
    with tile.TileContext(nc) as tc:
        from contextlib import ExitStack

        with ExitStack() as ctx:
            const_pool = ctx.enter_context(tc.tile_pool(name="const", bufs=1))
            xnat_pool = ctx.enter_context(tc.tile_pool(name="xnat", bufs=2))
            xt_pool = ctx.enter_context(tc.tile_pool(name="xt", bufs=1))
            w_pool = ctx.enter_context(tc.tile_pool(name="w", bufs=3))
            st_pool = ctx.enter_context(tc.tile_pool(name="st", bufs=2))
            pt_pool = ctx.enter_context(tc.tile_pool(name="pt", bufs=3, space="PSUM"))
            pm_pool = ctx.enter_context(tc.tile_pool(name="pm", bufs=4, space="PSUM"))

            ident = const_pool.tile([128, 128], F32, tag="ident")
            nc.sync.dma_start(ident[:], ident_ap[:])
            biasT = const_pool.tile([D, K], F32, tag="bias")
            nc.sync.dma_start(biasT[:], bias_ap[:])

            xT = [
                xt_pool.tile([128, BT], F32, tag=f"xT{j}", name=f"xT{j}")
                for j in range(n_chunks)
            ]

            # weight mega-loads: groups of WGRP chunks, (128, WGRP*128) tiles
            w3 = w_ap.rearrange("(g n) p d -> g p n d", n=WGRP)
            wtiles = [
                w_pool.tile([128, WGRP, D], F32, tag="wg", name=f"wg{g}")
                for g in range(n_wgrp)
            ]

            for b in range(B):
                xn = xnat_pool.tile([Tc, NF], F32, tag="xnat", name=f"xn{b}")
                if nfull < n_chunks:
                    nc.gpsimd.memset(xn[:, nfull * 128 : n_chunks * 128], 0.0)
                xn3 = xn.rearrange("p (j z) -> p j z", z=128)
                for c in range(C):
                    src = x_ap[b, c][:, : nfull * G].rearrange(
                        "p (j w) -> p j w", w=G
                    )
                    x_engines[c].dma_start(xn3[:, :nfull, c * G : (c + 1) * G], src)
                    for fi in range(nfull * G, F):
                        j = fi // G
                        w = fi - G * j
                        nc.gpsimd.dma_start(
                            xn3[:, j, c * G + w : c * G + w + 1],
                            x_ap[b, c][:, fi : fi + 1],
                        )
                # weight mega-load g=b, after this b's x loads are queued
                if b < n_wgrp:
                    nc.sync.dma_start(wtiles[b][:], w3[b])
                for j in range(n_chunks):
                    pt = pt_pool.tile([128, Tc], F32, tag="pt")
                    nc.tensor.transpose(pt[:], xn3[:, j], ident[:])
                    if j % 2 == 0:
                        nc.vector.tensor_copy(
                            out=xT[j][:, b * Tc : (b + 1) * Tc], in_=pt[:]
                        )
                    else:
                        nc.scalar.copy(
                            out=xT[j][:, b * Tc : (b + 1) * Tc], in_=pt[:]
                        )

            for g in range(B, n_wgrp):
                nc.sync.dma_start(wtiles[g][:], w3[g])

            inc = 0
            st = None
            for k in range(K):
                chunks = band_chunks[k]
                pm = pm_pool.tile([D, BT], F32, tag="pm")
                for idx, j in enumerate(chunks):
                    nc.tensor.matmul(
                        pm[:],
                        wtiles[inc // WGRP][:, inc % WGRP],
                        xT[j][:],
                        start=(idx == 0),
                        stop=(idx == len(chunks) - 1),
                    )
                    inc += 1
                if k % KGRP == 0:
                    st = st_pool.tile([D, KGRP * BT], F32, tag="st")
                nc.scalar.activation(
                    st[:, (k % KGRP) * BT : (k % KGRP + 1) * BT],
                    pm[:],
                    mybir.ActivationFunctionType.Identity,
                    bias=biasT[:, k : k + 1],
                )
                if k % KGRP == KGRP - 1:
                    k0 = k - (KGRP - 1)
                    dst = out_ap[k0 : k0 + KGRP].rearrange("k d t -> d k t")
                    nc.gpsimd.dma_start(
                        dst, st.rearrange("d (k t) -> d k t", k=KGRP)
                    )

    nc.compile()
    return nc


def kernel(x, pre_w, pre_b, indices_pad, mask):
    x = np.asarray(x, np.float32)
    pre_w = np.asarray(pre_w, np.float32)
    pre_b = np.asarray(pre_b, np.float32)
    indices_pad = np.asarray(indices_pad)
    mask_np = np.asarray(mask)

    B, C, T, F = x.shape
    K, CW, D = pre_w.shape
    starts, ends = _band_ranges(indices_pad, mask_np)
    Wmax = CW // C
    n_chunks = (F + G - 1) // G  # 33
    assert T % N_CORES == 0
    Tc = T // N_CORES

    wchunks, incid = _pack_weights(pre_w, starts, ends, Wmax)
    n_inc = len(incid)
    band_chunks = [[] for _ in range(K)]
    for k, j in incid:
        band_chunks[k].append(j)
    # pad weight array to a multiple of WGRP chunks
    n_wgrp = (n_inc + WGRP - 1) // WGRP
    if n_wgrp * WGRP > n_inc:
        pad = np.zeros((n_wgrp * WGRP - n_inc, 128, D), np.float32)
        wchunks = np.concatenate([wchunks, pad], axis=0)

    key = (B, C, T, F, K, D, tuple(starts.tolist()), tuple(ends.tolist()))
    if key not in _cache:
        _cache[key] = _build_program(Tc, F, n_chunks, band_chunks, n_inc)
    nc = _cache[key]

    biasT = np.ascontiguousarray(pre_b.T)
    ident = np.eye(128, dtype=np.float32)
    in_maps = []
    for core in range(N_CORES):
        xs = np.ascontiguousarray(x[:, :, core * Tc : (core + 1) * Tc, :])
        in_maps.append({"x": xs, "w": wchunks, "bias": biasT, "ident": ident})

    global _last_in_maps
    _last_in_maps = in_maps
    res = run_bass_kernel_spmd(nc, in_maps, list(range(N_CORES)))

    # per-core out: (K, D, B*Tc) -> full (B, D, T, K)
    arr = np.stack([res.results[i]["out"] for i in range(N_CORES)])
    arr = arr.reshape(N_CORES, K, D, B, Tc)
    out = np.transpose(arr, (3, 2, 0, 4, 1)).reshape(B, D, T, K)
    return np.ascontiguousarray(out)


# revision 14
# speedup vs baseline: 1.1748x; 1.1333x over previous
"""BandSplit Trainium2 kernel.

out[b, d, t, k] = sum_{c,w} x[b, c, t, idx[k, w]] * pre_w[k, c*W + w, d] + pre_b[k, d]

Bands are contiguous frequency ranges (mel triangles), so the ragged
gather is a banded sparse matmul. Strategy (8 cores, shard T 8-way):

  * Each core takes t in [core*128, core*128+128) for all 4 batches
    (512 (b,t) rows).
  * x is loaded as (t=128 partitions, 33 chunks x [c*32 + w] cols) --
    the load DMA interleaves the 4 channels at 32-frequency-column
    granularity. PE transpose of each 128x128 block then yields
    xT[chunk] = (4c*32f = 128 partitions, 4b*128t = 512 free): the
    matmul contraction layout.
  * Weights are host-prepacked per (band, 32-aligned f-chunk) incidence
    into 128x128 fp32 tiles (zero rows outside the band) whose row
    order matches xT partitions. Per band: accumulate its incidences
    into one PSUM (d=128, 512) tile, fused bias-add on ScalarE into a
    grouped stage tile, DMA groups of 8 bands to a device-friendly
    (K, D, B*128) output. Host reassembles.
"""

import numpy as np

import concourse.bass as bass
import concourse.tile as tile
from concourse import bacc, mybir
from concourse.bass_utils import run_bass_kernel_spmd

F32 = mybir.dt.float32

N_CORES = 8
G = 32  # channel-interleave granularity (f columns per c segment)
WGRP = 32  # weight chunks per mega-DMA
KGRP = 8  # bands per output stage tile / DMA

_cache = {}


def _band_ranges(indices_pad, mask):
    W = mask.sum(axis=1).astype(np.int64)
    starts = indices_pad[:, 0].astype(np.int64)
    ends = starts + W
    return starts, ends


def _pack_weights(pre_w, starts, ends, Wmax):
    """(NINC, 128, 128) fp32 tiles; rows p = c*G + (f - 32*j) matching xT."""
    K, CW, D = pre_w.shape
    C = CW // Wmax
    pw = pre_w.reshape(K, C, Wmax, D)
    incid = []  # (k, j) pairs in band-major order
    for k in range(K):
        for j in range(int(starts[k]) // G, (int(ends[k]) - 1) // G + 1):
            incid.append((k, j))
    wchunks = np.zeros((len(incid), 128, D), np.float32)
    for i, (k, j) in enumerate(incid):
        f0 = max(int(starts[k]), G * j)
        f1 = min(int(ends[k]), G * (j + 1))
        w0 = f0 - int(starts[k])
        p0 = f0 - G * j
        n = f1 - f0
        for c in range(C):
            wchunks[i, c * G + p0 : c * G + p0 + n, :] = pw[k, c, w0 : w0 + n, :]
    return wchunks, incid


def _build_program(Tc, F, n_chunks, band_chunks, n_inc):
    """Bass program for one core. Tc = timesteps per core (128)."""
    B, C, K, D = 4, 4, 64, 128
    NF = n_chunks * 128  # xnat free size
    BT = B * Tc
    n_wgrp = (n_inc + WGRP - 1) // WGRP

    nc = bacc.Bacc("TRN2", target_bir_lowering=False, debug=False)
    # x is host-pre-interleaved: (B, Tc, n_chunks*128) with col j*128 + c*32 + w
    x_ap = nc.dram_tensor(
        "x", [B, Tc, n_chunks * 128], F32, kind="ExternalInput"
    ).ap()
    w_ap = nc.dram_tensor("w", [128, n_wgrp * WGRP * D], F32, kind="ExternalInput").ap()
    bias_ap = nc.dram_tensor("bias", [D, K], F32, kind="ExternalInput").ap()
    ident_ap = nc.dram_tensor("ident", [128, 128], F32, kind="ExternalInput").ap()
    out_ap = nc.dram_tensor("out", [K, D, BT], F32, kind="ExternalOutput").ap()

    # DMA-issuing engines for the x interleave loads, round-robin
    x_engines = [nc.sync, nc.scalar]

    nfull = F // G  # full 32-col f chunks (32)
    with tile.TileContext(nc) as tc:
        from contextlib import ExitStack

        with ExitStack() as ctx:
            const_pool = ctx.enter_context(tc.tile_pool(name="const", bufs=1))
            xnat_pool = ctx.enter_context(tc.tile_pool(name="xnat", bufs=2))
            xt_pool = ctx.enter_context(tc.tile_pool(name="xt", bufs=1))
            w_pool = ctx.enter_context(tc.tile_pool(name="w", bufs=3))
            st_pool = ctx.enter_context(tc.tile_pool(name="st", bufs=2))
            pt_pool = ctx.enter_context(tc.tile_pool(name="pt", bufs=3, space="PSUM"))
            pm_pool = ctx.enter_context(tc.tile_pool(name="pm", bufs=4, space="PSUM"))

            ident = const_pool.tile([128, 128], F32, tag="ident")
            nc.sync.dma_start(ident[:], ident_ap[:])
            biasT = const_pool.tile([D, K], F32, tag="bias")
            nc.sync.dma_start(biasT[:], bias_ap[:])

            xT = [
                xt_pool.tile([128, BT], F32, tag=f"xT{j}", name=f"xT{j}")
                for j in range(n_chunks)
            ]

            # weight mega-loads: host layout is partition-major
            # (128, n_wgrp*WGRP*128) so each load is contiguous per partition
            wtiles = [
                w_pool.tile([128, WGRP * D], F32, tag="wg", name=f"wg{g}")
                for g in range(n_wgrp)
            ]

            for b in range(B):
                xn = xnat_pool.tile([Tc, NF], F32, tag="xnat", name=f"xn{b}")
                x_engines[b % 2].dma_start(xn[:], x_ap[b])
                # weight mega-load g=b, after this b's x load is queued
                if b < n_wgrp:
                    nc.sync.dma_start(
                        wtiles[b][:],
                        w_ap[:, b * WGRP * D : (b + 1) * WGRP * D],
                    )
                for j in range(n_chunks):
                    pt = pt_pool.tile([128, Tc], F32, tag="pt")
                    nc.tensor.transpose(
                        pt[:], xn[:, j * 128 : (j + 1) * 128], ident[:]
                    )
                    if j % 2 == 0:
                        nc.vector.tensor_copy(
                            out=xT[j][:, b * Tc : (b + 1) * Tc], in_=pt[:]
                        )
                    else:
                        nc.scalar.copy(
                            out=xT[j][:, b * Tc : (b + 1) * Tc], in_=pt[:]
                        )

            for g in range(B, n_wgrp):
                nc.sync.dma_start(
                    wtiles[g][:], w_ap[:, g * WGRP * D : (g + 1) * WGRP * D]
                )

            inc = 0
            st = None
            for k in range(K):
                chunks = band_chunks[k]
                pm = pm_pool.tile([D, BT], F32, tag="pm")
                for idx, j in enumerate(chunks):
                    nc.tensor.matmul(
                        pm[:],
                        wtiles[inc // WGRP][:, (inc % WGRP) * D : (inc % WGRP + 1) * D],
                        xT[j][:],
                        start=(idx == 0),
                        stop=(idx == len(chunks) - 1),
                    )
                    inc += 1
                if k % KGRP == 0:
                    st = st_pool.tile([D, KGRP * BT], F32, tag="st")
                nc.scalar.activation(
                    st[:, (k % KGRP) * BT : (k % KGRP + 1) * BT],
                    pm[:],
                    mybir.ActivationFunctionType.Identity,
                    bias=biasT[:, k : k + 1],
                )
                if k % KGRP == KGRP - 1:
                    k0 = k - (KGRP - 1)
                    dst = out_ap[k0 : k0 + KGRP].rearrange("k d t -> d k t")
                    nc.gpsimd.dma_start(
                        dst, st.rearrange("d (k t) -> d k t", k=KGRP)
                    )

    nc.compile()
    return nc


def kernel(x, pre_w, pre_b, indices_pad, mask):
    x = np.asarray(x, np.float32)
    pre_w = np.asarray(pre_w, np.float32)
    pre_b = np.asarray(pre_b, np.float32)
    indices_pad = np.asarray(indices_pad)
    mask_np = np.asarray(mask)

    B, C, T, F = x.shape
    K, CW, D = pre_w.shape
    starts, ends = _band_ranges(indices_pad, mask_np)
    Wmax = CW // C
    n_chunks = (F + G - 1) // G  # 33
    assert T % N_CORES == 0
    Tc = T // N_CORES

    wchunks, incid = _pack_weights(pre_w, starts, ends, Wmax)
    n_inc = len(incid)
    band_chunks = [[] for _ in range(K)]
    for k, j in incid:
        band_chunks[k].append(j)
    # pad weight array to a multiple of WGRP chunks; partition-major
    # layout (128, NINC*128) so device loads are contiguous per partition
    n_wgrp = (n_inc + WGRP - 1) // WGRP
    if n_wgrp * WGRP > n_inc:
        pad = np.zeros((n_wgrp * WGRP - n_inc, 128, D), np.float32)
        wchunks = np.concatenate([wchunks, pad], axis=0)
    wchunks = np.ascontiguousarray(wchunks.transpose(1, 0, 2).reshape(128, -1))

    key = (B, C, T, F, K, D, tuple(starts.tolist()), tuple(ends.tolist()))
    if key not in _cache:
        _cache[key] = _build_program(Tc, F, n_chunks, band_chunks, n_inc)
    nc = _cache[key]

    biasT = np.ascontiguousarray(pre_b.T)
    ident = np.eye(128, dtype=np.float32)

    # host interleave: xi[b, t, j*128 + c*32 + w] = x[b, c, t, 32j + w]
    FQ = n_chunks * G
    xpad = np.zeros((B, C, T, FQ), np.float32)
    xpad[:, :, :, :F] = x
    xi = np.ascontiguousarray(
        xpad.reshape(B, C, T, n_chunks, G).transpose(0, 2, 3, 1, 4)
    ).reshape(B, T, n_chunks * C * G)

    in_maps = []
    for core in range(N_CORES):
        xs = np.ascontiguousarray(xi[:, core * Tc : (core + 1) * Tc, :])
        in_maps.append({"x": xs, "w": wchunks, "bias": biasT, "ident": ident})

    global _last_in_maps
    _last_in_maps = in_maps
    res = run_bass_kernel_spmd(nc, in_maps, list(range(N_CORES)))

    # per-core out: (K, D, B*Tc) -> full (B, D, T, K)
    arr = np.stack([res.results[i]["out"] for i in range(N_CORES)])
    arr = arr.reshape(N_CORES, K, D, B, Tc)
    out = np.transpose(arr, (3, 2, 0, 4, 1)).reshape(B, D, T, K)
    return np.ascontiguousarray(out)


# revision 16
# speedup vs baseline: 1.2376x; 1.0534x over previous
"""BandSplit Trainium2 kernel.

out[b, d, t, k] = sum_{c,w} x[b, c, t, idx[k, w]] * pre_w[k, c*W + w, d] + pre_b[k, d]

Bands are contiguous frequency ranges (mel triangles), so the ragged
gather is a banded sparse matmul. Strategy (8 cores, shard T 8-way):

  * Each core takes t in [core*128, core*128+128) for all 4 batches
    (512 (b,t) rows).
  * The host pre-interleaves x into (B, T, 33 chunks x [c*32 + w]) and
    splits fp32 into bf16 hi + bf16 lo halves. On device, PE transposes
    of each 128x128 block yield xT[chunk] = (4c*32f = 128 partitions,
    4b*128t = 512 free): the matmul contraction layout.
  * Weights are host-prepacked per (band, 32-aligned f-chunk) incidence
    into 128x128 tiles (zero rows outside the band) matching the xT row
    order, split into bf16 hi/lo, and stored partition-major so device
    loads are contiguous. Per band: accumulate 3 bf16 matmuls per
    incidence (Xh@Wh + Xh@Wl + Xl@Wh, fp32 PSUM) -- full-fp32-quality
    at 3/4 the fp32 PE streaming cost. Fused bias-add on ScalarE into a
    grouped stage tile, group-DMA to a device-friendly (K, D, B*128)
    output. Host reassembles.
"""

import ml_dtypes
import numpy as np

import concourse.bass as bass
import concourse.tile as tile
from concourse import bacc, mybir
from concourse.bass_utils import run_bass_kernel_spmd

F32 = mybir.dt.float32
BF16 = mybir.dt.bfloat16
NP_BF16 = ml_dtypes.bfloat16

N_CORES = 8
G = 32  # channel-interleave granularity (f columns per c segment)
WGRP = 32  # weight chunks per mega-DMA
KGRP = 8  # bands per output stage tile / DMA
XSPLIT = 4  # sub-DMAs per x load (prefix latency)

_cache = {}


def _band_ranges(indices_pad, mask):
    W = mask.sum(axis=1).astype(np.int64)
    starts = indices_pad[:, 0].astype(np.int64)
    ends = starts + W
    return starts, ends


def _pack_weights(pre_w, starts, ends, Wmax):
    """(NINC, 128, 128) fp32 tiles; rows p = c*G + (f - 32*j) matching xT."""
    K, CW, D = pre_w.shape
    C = CW // Wmax
    pw = pre_w.reshape(K, C, Wmax, D)
    incid = []  # (k, j) pairs in band-major order
    for k in range(K):
        for j in range(int(starts[k]) // G, (int(ends[k]) - 1) // G + 1):
            incid.append((k, j))
    wchunks = np.zeros((len(incid), 128, D), np.float32)
    for i, (k, j) in enumerate(incid):
        f0 = max(int(starts[k]), G * j)
        f1 = min(int(ends[k]), G * (j + 1))
        w0 = f0 - int(starts[k])
        p0 = f0 - G * j
        n = f1 - f0
        for c in range(C):
            wchunks[i, c * G + p0 : c * G + p0 + n, :] = pw[k, c, w0 : w0 + n, :]
    return wchunks, incid


def _build_program(Tc, n_chunks, band_chunks, n_inc):
    """Bass program for one core. Tc = timesteps per core (128)."""
    B, C, K, D = 4, 4, 64, 128
    NF = n_chunks * 128  # interleaved cols (4224)
    BT = B * Tc
    n_wgrp = (n_inc + WGRP - 1) // WGRP

    nc = bacc.Bacc("TRN2", target_bir_lowering=False, debug=False)
    # x: host-interleaved (B, Tc, 33*128), col j*128 + c*32 + w; bf16 hi/lo
    xh_ap = nc.dram_tensor("xh", [B, Tc, NF], BF16, kind="ExternalInput").ap()
    xl_ap = nc.dram_tensor("xl", [B, Tc, NF], BF16, kind="ExternalInput").ap()
    # weights: partition-major (128, NINC*128), bf16 hi/lo
    wh_ap = nc.dram_tensor(
        "wh", [128, n_wgrp * WGRP * D], BF16, kind="ExternalInput"
    ).ap()
    wl_ap = nc.dram_tensor(
        "wl", [128, n_wgrp * WGRP * D], BF16, kind="ExternalInput"
    ).ap()
    bias_ap = nc.dram_tensor("bias", [D, K], F32, kind="ExternalInput").ap()
    ident_ap = nc.dram_tensor("ident", [128, 128], BF16, kind="ExternalInput").ap()
    out_ap = nc.dram_tensor("out", [K, D, BT], F32, kind="ExternalOutput").ap()

    with tile.TileContext(nc) as tc:
        from contextlib import ExitStack

        with ExitStack() as ctx:
            const_pool = ctx.enter_context(tc.tile_pool(name="const", bufs=1))
            xnat_pool = ctx.enter_context(tc.tile_pool(name="xnat", bufs=2))
            xt_pool = ctx.enter_context(tc.tile_pool(name="xt", bufs=1))
            w_pool = ctx.enter_context(tc.tile_pool(name="w", bufs=4))
            st_pool = ctx.enter_context(tc.tile_pool(name="st", bufs=2))
            pt_pool = ctx.enter_context(tc.tile_pool(name="pt", bufs=4, space="PSUM"))
            pm_pool = ctx.enter_context(tc.tile_pool(name="pm", bufs=4, space="PSUM"))

            ident = const_pool.tile([128, 128], BF16, tag="ident")
            nc.sync.dma_start(ident[:], ident_ap[:])
            biasT = const_pool.tile([D, K], F32, tag="bias")
            nc.sync.dma_start(biasT[:], bias_ap[:])

            xT = {}
            for h in range(2):
                for j in range(n_chunks):
                    xT[h, j] = xt_pool.tile(
                        [128, BT], BF16, tag=f"xT{h}_{j}", name=f"xT{h}_{j}"
                    )

            # weight mega-loads, interleaved hi/lo groups
            wtiles = {}
            for g in range(n_wgrp):
                for h in range(2):
                    wtiles[h, g] = w_pool.tile(
                        [128, WGRP * D], BF16, tag="wg", name=f"wg{h}_{g}"
                    )

            def load_wgroup(g):
                for h, ap in ((0, wh_ap), (1, wl_ap)):
                    nc.sync.dma_start(
                        wtiles[h, g][:], ap[:, g * WGRP * D : (g + 1) * WGRP * D]
                    )

            # chunk ranges per x sub-DMA piece
            bounds = [n_chunks * p // XSPLIT for p in range(XSPLIT + 1)]

            for b in range(B):
                xn = {}
                for h, ap in ((0, xh_ap), (1, xl_ap)):
                    xn[h] = xnat_pool.tile(
                        [Tc, NF], BF16, tag=f"xnat{h}", name=f"xn{h}_{b}"
                    )
                    for p in range(XSPLIT):
                        c0, c1 = bounds[p] * 128, bounds[p + 1] * 128
                        eng = nc.sync if (p + h) % 2 == 0 else nc.scalar
                        eng.dma_start(xn[h][:, c0:c1], ap[b][:, c0:c1])
                if b < n_wgrp:
                    load_wgroup(b)
                for j in range(n_chunks):
                    for h in range(2):
                        pt = pt_pool.tile([128, Tc], BF16, tag="pt")
                        nc.tensor.transpose(
                            pt[:], xn[h][:, j * 128 : (j + 1) * 128], ident[:]
                        )
                        if (j + h) % 2 == 0:
                            nc.vector.tensor_copy(
                                out=xT[h, j][:, b * Tc : (b + 1) * Tc], in_=pt[:]
                            )
                        else:
                            nc.scalar.copy(
                                out=xT[h, j][:, b * Tc : (b + 1) * Tc], in_=pt[:]
                            )

            for g in range(B, n_wgrp):
                load_wgroup(g)

            inc = 0
            st = None
            for k in range(K):
                chunks = band_chunks[k]
                pm = pm_pool.tile([D, BT], F32, tag="pm")
                last = len(chunks) * 3 - 1
                step = 0
                for j in chunks:
                    g, i = inc // WGRP, inc % WGRP
                    for wt, xt in (
                        (wtiles[0, g], xT[0, j]),
                        (wtiles[1, g], xT[0, j]),
                        (wtiles[0, g], xT[1, j]),
                    ):
                        nc.tensor.matmul(
                            pm[:],
                            wt[:, i * D : (i + 1) * D],
                            xt[:],
                            start=(step == 0),
                            stop=(step == last),
                        )
                        step += 1
                    inc += 1
                if k % KGRP == 0:
                    st = st_pool.tile([D, KGRP * BT], F32, tag="st")
                nc.scalar.activation(
                    st[:, (k % KGRP) * BT : (k % KGRP + 1) * BT],
                    pm[:],
                    mybir.ActivationFunctionType.Identity,
                    bias=biasT[:, k : k + 1],
                )
                if k % KGRP == KGRP - 1:
                    k0 = k - (KGRP - 1)
                    dst = out_ap[k0 : k0 + KGRP].rearrange("k d t -> d k t")
                    nc.sync.dma_start(
                        dst, st.rearrange("d (k t) -> d k t", k=KGRP)
                    )

    nc.compile()
    return nc


def _split_bf16(a):
    hi = a.astype(NP_BF16)
    lo = (a - hi.astype(np.float32)).astype(NP_BF16)
    return hi, lo


def kernel(x, pre_w, pre_b, indices_pad, mask):
    x = np.asarray(x, np.float32)
    pre_w = np.asarray(pre_w, np.float32)
    pre_b = np.asarray(pre_b, np.float32)
    indices_pad = np.asarray(indices_pad)
    mask_np = np.asarray(mask)

    B, C, T, F = x.shape
    K, CW, D = pre_w.shape
    starts, ends = _band_ranges(indices_pad, mask_np)
    Wmax = CW // C
    n_chunks = (F + G - 1) // G  # 33
    assert T % N_CORES == 0
    Tc = T // N_CORES

    wchunks, incid = _pack_weights(pre_w, starts, ends, Wmax)
    n_inc = len(incid)
    band_chunks = [[] for _ in range(K)]
    for k, j in incid:
        band_chunks[k].append(j)
    # pad to a multiple of WGRP chunks; partition-major (128, NINC*128)
    n_wgrp = (n_inc + WGRP - 1) // WGRP
    if n_wgrp * WGRP > n_inc:
        pad = np.zeros((n_wgrp * WGRP - n_inc, 128, D), np.float32)
        wchunks = np.concatenate([wchunks, pad], axis=0)
    wflat = np.ascontiguousarray(wchunks.transpose(1, 0, 2).reshape(128, -1))
    wh, wl = _split_bf16(wflat)

    key = (B, C, T, F, K, D, tuple(starts.tolist()), tuple(ends.tolist()))
    if key not in _cache:
        _cache[key] = _build_program(Tc, n_chunks, band_chunks, n_inc)
    nc = _cache[key]

    biasT = np.ascontiguousarray(pre_b.T)
    ident = np.eye(128, dtype=NP_BF16)

    # host interleave: xi[b, t, j*128 + c*32 + w] = x[b, c, t, 32j + w]
    FQ = n_chunks * G
    xpad = np.zeros((B, C, T, FQ), np.float32)
    xpad[:, :, :, :F] = x
    xi = np.ascontiguousarray(
        xpad.reshape(B, C, T, n_chunks, G).transpose(0, 2, 3, 1, 4)
    ).reshape(B, T, n_chunks * C * G)
    xih, xil = _split_bf16(xi)

    in_maps = []
    for core in range(N_CORES):
        sl = slice(core * Tc, (core + 1) * Tc)
        in_maps.append(
            {
                "xh": np.ascontiguousarray(xih[:, sl]),
                "xl": np.ascontiguousarray(xil[:, sl]),
                "wh": wh,
                "wl": wl,
                "bias": biasT,
                "ident": ident,
            }
        )

    global _last_in_maps
    _last_in_maps = in_maps
    res = run_bass_kernel_spmd(nc, in_maps, list(range(N_CORES)))

    # per-core out: (K, D, B*Tc) -> full (B, D, T, K)
    arr = np.stack([res.results[i]["out"] for i in range(N_CORES)])
    arr = arr.reshape(N_CORES, K, D, B, Tc)
    out = np.transpose(arr, (3, 2, 0, 4, 1)).reshape(B, D, T, K)
    return np.ascontiguousarray(out)


# revision 18
# speedup vs baseline: 1.4164x; 1.1445x over previous
"""BandSplit Trainium2 kernel.

out[b, d, t, k] = sum_{c,w} x[b, c, t, idx[k, w]] * pre_w[k, c*W + w, d] + pre_b[k, d]

Bands are contiguous frequency ranges (mel triangles), so the ragged
gather is a banded sparse matmul. Strategy (8 cores, shard T 8-way):

  * Each core takes t in [core*128, core*128+128) for all 4 batches
    (512 (b,t) rows).
  * The host pre-interleaves x into (B, T, 33 chunks x [c*32 + w]) and
    splits fp32 into bf16 hi + bf16 lo halves. On device, PE transposes
    of each 128x128 block yield xT[chunk] = (4c*32f = 128 partitions,
    4b*128t = 512 free): the matmul contraction layout.
  * Weights are host-prepacked per (band, 32-aligned f-chunk) incidence
    into 128x128 tiles (zero rows outside the band) matching the xT row
    order, split into bf16 hi/lo, and stored partition-major so device
    loads are contiguous. Per band: accumulate 3 bf16 matmuls per
    incidence (Xh@Wh + Xh@Wl + Xl@Wh, fp32 PSUM) -- full-fp32-quality
    at 3/4 the fp32 PE streaming cost. Fused bias-add on ScalarE into a
    grouped stage tile, group-DMA to a device-friendly (K, D, B*128)
    output. Host reassembles.
"""

import ml_dtypes
import numpy as np

import concourse.bass as bass
import concourse.tile as tile
from concourse import bacc, mybir
from concourse.bass_utils import run_bass_kernel_spmd

F32 = mybir.dt.float32
BF16 = mybir.dt.bfloat16
NP_BF16 = ml_dtypes.bfloat16

N_CORES = 8
G = 32  # channel-interleave granularity (f columns per c segment)
WGRP = 32  # weight chunks per mega-DMA
KGRP = 4  # bands per output stage tile / DMA
XSPLIT = 4  # sub-DMAs per x load (prefix latency)

_cache = {}


def _band_ranges(indices_pad, mask):
    W = mask.sum(axis=1).astype(np.int64)
    starts = indices_pad[:, 0].astype(np.int64)
    ends = starts + W
    return starts, ends


def _pack_weights(pre_w, starts, ends, Wmax):
    """(NINC, 128, 128) fp32 tiles; rows p = c*G + (f - 32*j) matching xT."""
    K, CW, D = pre_w.shape
    C = CW // Wmax
    pw = pre_w.reshape(K, C, Wmax, D)
    incid = []  # (k, j) pairs in band-major order
    for k in range(K):
        for j in range(int(starts[k]) // G, (int(ends[k]) - 1) // G + 1):
            incid.append((k, j))
    wchunks = np.zeros((len(incid), 128, D), np.float32)
    for i, (k, j) in enumerate(incid):
        f0 = max(int(starts[k]), G * j)
        f1 = min(int(ends[k]), G * (j + 1))
        w0 = f0 - int(starts[k])
        p0 = f0 - G * j
        n = f1 - f0
        for c in range(C):
            wchunks[i, c * G + p0 : c * G + p0 + n, :] = pw[k, c, w0 : w0 + n, :]
    return wchunks, incid


def _build_program(Tc, n_chunks, band_chunks, n_inc):
    """Bass program for one core. Tc = timesteps per core (128)."""
    B, C, K, D = 4, 4, 64, 128
    NF = n_chunks * 128  # interleaved cols (4224)
    BT = B * Tc
    n_wgrp = (n_inc + WGRP - 1) // WGRP

    nc = bacc.Bacc("TRN2", target_bir_lowering=False, debug=False)
    # x: host-interleaved (B, Tc, 33*128), col j*128 + c*32 + w; bf16 hi/lo
    xh_ap = nc.dram_tensor("xh", [B, Tc, NF], BF16, kind="ExternalInput").ap()
    xl_ap = nc.dram_tensor("xl", [B, Tc, NF], BF16, kind="ExternalInput").ap()
    # weights: partition-major (128, NINC*128), bf16 hi/lo
    wh_ap = nc.dram_tensor(
        "wh", [128, n_wgrp * WGRP * D], BF16, kind="ExternalInput"
    ).ap()
    wl_ap = nc.dram_tensor(
        "wl", [128, n_wgrp * WGRP * D], BF16, kind="ExternalInput"
    ).ap()
    bias_ap = nc.dram_tensor("bias", [D, K], F32, kind="ExternalInput").ap()
    ident_ap = nc.dram_tensor("ident", [128, 128], BF16, kind="ExternalInput").ap()
    out_ap = nc.dram_tensor("out", [K, D, BT], F32, kind="ExternalOutput").ap()

    with tile.TileContext(nc) as tc:
        from contextlib import ExitStack

        with ExitStack() as ctx:
            const_pool = ctx.enter_context(tc.tile_pool(name="const", bufs=1))
            xnat_pool = ctx.enter_context(tc.tile_pool(name="xnat", bufs=4))
            xt_pool = ctx.enter_context(tc.tile_pool(name="xt", bufs=1))
            w_pool = ctx.enter_context(tc.tile_pool(name="w", bufs=4))
            st_pool = ctx.enter_context(tc.tile_pool(name="st", bufs=2))
            pt_pool = ctx.enter_context(tc.tile_pool(name="pt", bufs=4, space="PSUM"))
            pm_pool = ctx.enter_context(tc.tile_pool(name="pm", bufs=4, space="PSUM"))

            ident = const_pool.tile([128, 128], BF16, tag="ident")
            nc.sync.dma_start(ident[:], ident_ap[:])
            biasT = const_pool.tile([D, K], F32, tag="bias")
            nc.sync.dma_start(biasT[:], bias_ap[:])

            xT = {}
            for h in range(2):
                for j in range(n_chunks):
                    xT[h, j] = xt_pool.tile(
                        [128, BT], BF16, tag=f"xT{h}_{j}", name=f"xT{h}_{j}"
                    )

            # weight mega-loads, interleaved hi/lo groups
            wtiles = {}
            for g in range(n_wgrp):
                for h in range(2):
                    wtiles[h, g] = w_pool.tile(
                        [128, WGRP * D], BF16, tag="wg", name=f"wg{h}_{g}"
                    )

            def load_wgroup(g):
                for h, ap in ((0, wh_ap), (1, wl_ap)):
                    nc.sync.dma_start(
                        wtiles[h, g][:], ap[:, g * WGRP * D : (g + 1) * WGRP * D]
                    )

            # chunk ranges per x sub-DMA piece
            bounds = [n_chunks * p // XSPLIT for p in range(XSPLIT + 1)]

            # all x loads upfront (bufs=4 holds all batches); pieces spread
            # across queues so transfers run concurrently
            xn = {}
            for b in range(B):
                for h, ap in ((0, xh_ap), (1, xl_ap)):
                    xn[h, b] = xnat_pool.tile(
                        [Tc, NF], BF16, tag=f"xnat{h}", name=f"xn{h}_{b}"
                    )
                    for p in range(XSPLIT):
                        c0, c1 = bounds[p] * 128, bounds[p + 1] * 128
                        eng = nc.sync if (p + h + b) % 2 == 0 else nc.scalar
                        eng.dma_start(xn[h, b][:, c0:c1], ap[b][:, c0:c1])
                if b < n_wgrp:
                    load_wgroup(b)
            for g in range(B, n_wgrp):
                load_wgroup(g)

            # j-major transposes: xT[:, j] completes early so band matmuls
            # can start while later chunks still transpose
            for j in range(n_chunks):
                for b in range(B):
                    for h in range(2):
                        pt = pt_pool.tile([128, Tc], BF16, tag="pt")
                        nc.tensor.transpose(
                            pt[:], xn[h, b][:, j * 128 : (j + 1) * 128], ident[:]
                        )
                        nc.vector.tensor_copy(
                            out=xT[h, j][:, b * Tc : (b + 1) * Tc], in_=pt[:]
                        )

            inc = 0
            st = None
            for k in range(K):
                chunks = band_chunks[k]
                pm = pm_pool.tile([D, BT], F32, tag="pm")
                last = len(chunks) * 3 - 1
                step = 0
                for j in chunks:
                    g, i = inc // WGRP, inc % WGRP
                    for wt, xt in (
                        (wtiles[0, g], xT[0, j]),
                        (wtiles[1, g], xT[0, j]),
                        (wtiles[0, g], xT[1, j]),
                    ):
                        nc.tensor.matmul(
                            pm[:],
                            wt[:, i * D : (i + 1) * D],
                            xt[:],
                            start=(step == 0),
                            stop=(step == last),
                        )
                        step += 1
                    inc += 1
                if k % KGRP == 0:
                    st = st_pool.tile([D, KGRP * BT], F32, tag="st")
                nc.scalar.activation(
                    st[:, (k % KGRP) * BT : (k % KGRP + 1) * BT],
                    pm[:],
                    mybir.ActivationFunctionType.Identity,
                    bias=biasT[:, k : k + 1],
                )
                if k % KGRP == KGRP - 1:
                    k0 = k - (KGRP - 1)
                    dst = out_ap[k0 : k0 + KGRP].rearrange("k d t -> d k t")
                    nc.sync.dma_start(
                        dst, st.rearrange("d (k t) -> d k t", k=KGRP)
                    )

    nc.compile()
    return nc


def _split_bf16(a):
    hi = a.astype(NP_BF16)
    lo = (a - hi.astype(np.float32)).astype(NP_BF16)
    return hi, lo


def kernel(x, pre_w, pre_b, indices_pad, mask):
    x = np.asarray(x, np.float32)
    pre_w = np.asarray(pre_w, np.float32)
    pre_b = np.asarray(pre_b, np.float32)
    indices_pad = np.asarray(indices_pad)
    mask_np = np.asarray(mask)

    B, C, T, F = x.shape
    K, CW, D = pre_w.shape
    starts, ends = _band_ranges(indices_pad, mask_np)
    Wmax = CW // C
    n_chunks = (F + G - 1) // G  # 33
    assert T % N_CORES == 0
    Tc = T // N_CORES

    wchunks, incid = _pack_weights(pre_w, starts, ends, Wmax)
    n_inc = len(incid)
    band_chunks = [[] for _ in range(K)]
    for k, j in incid:
        band_chunks[k].append(j)
    # pad to a multiple of WGRP chunks; partition-major (128, NINC*128)
    n_wgrp = (n_inc + WGRP - 1) // WGRP
    if n_wgrp * WGRP > n_inc:
        pad = np.zeros((n_wgrp * WGRP - n_inc, 128, D), np.float32)
        wchunks = np.concatenate([wchunks, pad], axis=0)
    wflat = np.ascontiguousarray(wchunks.transpose(1, 0, 2).reshape(128, -1))
    wh, wl = _split_bf16(wflat)

    key = (B, C, T, F, K, D, tuple(starts.tolist()), tuple(ends.tolist()))
    if key not in _cache:
        _cache[key] = _build_program(Tc, n_chunks, band_chunks, n_inc)
    nc = _cache[key]

    biasT = np.ascontiguousarray(pre_b.T)
    ident = np.eye(128, dtype=NP_BF16)

    # host interleave: xi[b, t, j*128 + c*32 + w] = x[b, c, t, 32j + w]
    FQ = n_chunks * G
    xpad = np.zeros((B, C, T, FQ), np.float32)
    xpad[:, :, :, :F] = x
    xi = np.ascontiguousarray(
        xpad.reshape(B, C, T, n_chunks, G).transpose(0, 2, 3, 1, 4)
    ).reshape(B, T, n_chunks * C * G)
    xih, xil = _split_bf16(xi)

    in_maps = []
    for core in range(N_CORES):
        sl = slice(core * Tc, (core + 1) * Tc)
        in_maps.append(
            {
                "xh": np.ascontiguousarray(xih[:, sl]),
                "xl": np.ascontiguousarray(xil[:, sl]),
                "wh": wh,
                "wl": wl,
                "bias": biasT,
                "ident": ident,
            }
        )

    global _last_in_maps
    _last_in_maps = in_maps
    res = run_bass_kernel_spmd(nc, in_maps, list(range(N_CORES)))

    # per-core out: (K, D, B*Tc) -> full (B, D, T, K)
    arr = np.stack([res.results[i]["out"] for i in range(N_CORES)])
    arr = arr.reshape(N_CORES, K, D, B, Tc)
    out = np.transpose(arr, (3, 2, 0, 4, 1)).reshape(B, D, T, K)
    return np.ascontiguousarray(out)


# revision 19
# speedup vs baseline: 1.4662x; 1.0352x over previous
"""BandSplit Trainium2 kernel.

out[b, d, t, k] = sum_{c,w} x[b, c, t, idx[k, w]] * pre_w[k, c*W + w, d] + pre_b[k, d]

Bands are contiguous frequency ranges (mel triangles), so the ragged
gather is a banded sparse matmul. Strategy (8 cores, shard T 8-way):

  * Each core takes t in [core*128, core*128+128) for all 4 batches
    (512 (b,t) rows).
  * The host pre-interleaves x into (B, T, 33 chunks x [c*32 + w]) and
    splits fp32 into bf16 hi + bf16 lo halves. On device, PE transposes
    of each 128x128 block yield xT[chunk] = (4c*32f = 128 partitions,
    4b*128t = 512 free): the matmul contraction layout.
  * Weights are host-prepacked per (band, 32-aligned f-chunk) incidence
    into 128x128 tiles (zero rows outside the band) matching the xT row
    order, split into bf16 hi/lo, and stored partition-major so device
    loads are contiguous. Per band: accumulate 3 bf16 matmuls per
    incidence (Xh@Wh + Xh@Wl + Xl@Wh, fp32 PSUM) -- full-fp32-quality
    at 3/4 the fp32 PE streaming cost. Fused bias-add on ScalarE into a
    grouped stage tile, group-DMA to a device-friendly (K, D, B*128)
    output. Host reassembles.
"""

import ml_dtypes
import numpy as np

import concourse.bass as bass
import concourse.tile as tile
from concourse import bacc, mybir
from concourse.bass_utils import run_bass_kernel_spmd

F32 = mybir.dt.float32
BF16 = mybir.dt.bfloat16
NP_BF16 = ml_dtypes.bfloat16

N_CORES = 8
G = 32  # channel-interleave granularity (f columns per c segment)
WGRP = 32  # weight chunks per mega-DMA
KGRP = 4  # bands per output stage tile / DMA
XSPLIT = 4  # sub-DMAs per x load (prefix latency)

_cache = {}


def _band_ranges(indices_pad, mask):
    W = mask.sum(axis=1).astype(np.int64)
    starts = indices_pad[:, 0].astype(np.int64)
    ends = starts + W
    return starts, ends


def _pack_weights(pre_w, starts, ends, Wmax):
    """(NINC, 128, 128) fp32 tiles; rows p = c*G + (f - 32*j) matching xT."""
    K, CW, D = pre_w.shape
    C = CW // Wmax
    pw = pre_w.reshape(K, C, Wmax, D)
    incid = []  # (k, j) pairs in band-major order
    for k in range(K):
        for j in range(int(starts[k]) // G, (int(ends[k]) - 1) // G + 1):
            incid.append((k, j))
    wchunks = np.zeros((len(incid), 128, D), np.float32)
    for i, (k, j) in enumerate(incid):
        f0 = max(int(starts[k]), G * j)
        f1 = min(int(ends[k]), G * (j + 1))
        w0 = f0 - int(starts[k])
        p0 = f0 - G * j
        n = f1 - f0
        for c in range(C):
            wchunks[i, c * G + p0 : c * G + p0 + n, :] = pw[k, c, w0 : w0 + n, :]
    return wchunks, incid


def _build_program(Tc, n_chunks, band_chunks, n_inc):
    """Bass program for one core. Tc = timesteps per core (128)."""
    B, C, K, D = 4, 4, 64, 128
    NF = n_chunks * 128  # interleaved cols (4224)
    BT = B * Tc
    n_wgrp = (n_inc + WGRP - 1) // WGRP

    nc = bacc.Bacc("TRN2", target_bir_lowering=False, debug=False)
    # x: host-interleaved (B, Tc, 33*128), col j*128 + c*32 + w; bf16 hi/lo
    xh_ap = nc.dram_tensor("xh", [B, Tc, NF], BF16, kind="ExternalInput").ap()
    xl_ap = nc.dram_tensor("xl", [B, Tc, NF], BF16, kind="ExternalInput").ap()
    # weights: partition-major (128, NINC*128), bf16 hi/lo
    wh_ap = nc.dram_tensor(
        "wh", [128, n_wgrp * WGRP * D], BF16, kind="ExternalInput"
    ).ap()
    wl_ap = nc.dram_tensor(
        "wl", [128, n_wgrp * WGRP * D], BF16, kind="ExternalInput"
    ).ap()
    bias_ap = nc.dram_tensor("bias", [D, K], F32, kind="ExternalInput").ap()
    ident_ap = nc.dram_tensor("ident", [128, 128], BF16, kind="ExternalInput").ap()
    out_ap = nc.dram_tensor("out", [K, D, BT], F32, kind="ExternalOutput").ap()

    with tile.TileContext(nc) as tc:
        from contextlib import ExitStack

        with ExitStack() as ctx:
            const_pool = ctx.enter_context(tc.tile_pool(name="const", bufs=1))
            xnat_pool = ctx.enter_context(tc.tile_pool(name="xnat", bufs=4))
            xt_pool = ctx.enter_context(tc.tile_pool(name="xt", bufs=1))
            w_pool = ctx.enter_context(tc.tile_pool(name="w", bufs=4))
            st_pool = ctx.enter_context(tc.tile_pool(name="st", bufs=2))
            pt_pool = ctx.enter_context(tc.tile_pool(name="pt", bufs=4, space="PSUM"))
            pm_pool = ctx.enter_context(tc.tile_pool(name="pm", bufs=4, space="PSUM"))

            ident = const_pool.tile([128, 128], BF16, tag="ident")
            nc.sync.dma_start(ident[:], ident_ap[:])
            biasT = const_pool.tile([D, K], F32, tag="bias")
            nc.sync.dma_start(biasT[:], bias_ap[:])

            xT = {}
            for h in range(2):
                for j in range(n_chunks):
                    xT[h, j] = xt_pool.tile(
                        [128, BT], BF16, tag=f"xT{h}_{j}", name=f"xT{h}_{j}"
                    )

            # weight mega-loads, interleaved hi/lo groups
            wtiles = {}
            for g in range(n_wgrp):
                for h in range(2):
                    wtiles[h, g] = w_pool.tile(
                        [128, WGRP * D], BF16, tag="wg", name=f"wg{h}_{g}"
                    )

            def load_wgroup(g):
                for h, ap in ((0, wh_ap), (1, wl_ap)):
                    nc.sync.dma_start(
                        wtiles[h, g][:], ap[:, g * WGRP * D : (g + 1) * WGRP * D]
                    )

            # chunk ranges per x sub-DMA piece
            bounds = [n_chunks * p // XSPLIT for p in range(XSPLIT + 1)]

            # all x loads upfront (bufs=4 holds all batches); piece-major
            # issue order so every batch's piece-0 lands first, engines
            # rotated so transfers run on many queues concurrently
            xn = {}
            for b in range(B):
                for h in range(2):
                    xn[h, b] = xnat_pool.tile(
                        [Tc, NF], BF16, tag=f"xnat{h}", name=f"xn{h}_{b}"
                    )
            engs = [nc.sync, nc.scalar, nc.gpsimd]
            e = 0
            for p in range(XSPLIT):
                c0, c1 = bounds[p] * 128, bounds[p + 1] * 128
                for b in range(B):
                    for h, ap in ((0, xh_ap), (1, xl_ap)):
                        engs[e % 3].dma_start(xn[h, b][:, c0:c1], ap[b][:, c0:c1])
                        e += 1
                if p < n_wgrp:
                    load_wgroup(p)
            for g in range(XSPLIT, n_wgrp):
                load_wgroup(g)

            # j-major transposes: xT[:, j] completes early so band matmuls
            # can start while later chunks still transpose
            for j in range(n_chunks):
                for b in range(B):
                    for h in range(2):
                        pt = pt_pool.tile([128, Tc], BF16, tag="pt")
                        nc.tensor.transpose(
                            pt[:], xn[h, b][:, j * 128 : (j + 1) * 128], ident[:]
                        )
                        nc.vector.tensor_copy(
                            out=xT[h, j][:, b * Tc : (b + 1) * Tc], in_=pt[:]
                        )

            inc = 0
            st = None
            for k in range(K):
                chunks = band_chunks[k]
                pm = pm_pool.tile([D, BT], F32, tag="pm")
                last = len(chunks) * 3 - 1
                step = 0
                for j in chunks:
                    g, i = inc // WGRP, inc % WGRP
                    for wt, xt in (
                        (wtiles[0, g], xT[0, j]),
                        (wtiles[1, g], xT[0, j]),
                        (wtiles[0, g], xT[1, j]),
                    ):
                        nc.tensor.matmul(
                            pm[:],
                            wt[:, i * D : (i + 1) * D],
                            xt[:],
                            start=(step == 0),
                            stop=(step == last),
                        )
                        step += 1
                    inc += 1
                if k % KGRP == 0:
                    st = st_pool.tile([D, KGRP * BT], F32, tag="st")
                nc.scalar.activation(
                    st[:, (k % KGRP) * BT : (k % KGRP + 1) * BT],
                    pm[:],
                    mybir.ActivationFunctionType.Identity,
                    bias=biasT[:, k : k + 1],
                )
                if k % KGRP == KGRP - 1:
                    k0 = k - (KGRP - 1)
                    dst = out_ap[k0 : k0 + KGRP].rearrange("k d t -> d k t")
                    nc.sync.dma_start(
                        dst, st.rearrange("d (k t) -> d k t", k=KGRP)
                    )

    nc.compile()
    return nc


def _split_bf16(a):
    hi = a.astype(NP_BF16)
    lo = (a - hi.astype(np.float32)).astype(NP_BF16)
    return hi, lo


def kernel(x, pre_w, pre_b, indices_pad, mask):
    x = np.asarray(x, np.float32)
    pre_w = np.asarray(pre_w, np.float32)
    pre_b = np.asarray(pre_b, np.float32)
    indices_pad = np.asarray(indices_pad)
    mask_np = np.asarray(mask)

    B, C, T, F = x.shape
    K, CW, D = pre_w.shape
    starts, ends = _band_ranges(indices_pad, mask_np)
    Wmax = CW // C
    n_chunks = (F + G - 1) // G  # 33
    assert T % N_CORES == 0
    Tc = T // N_CORES

    wchunks, incid = _pack_weights(pre_w, starts, ends, Wmax)
    n_inc = len(incid)
    band_chunks = [[] for _ in range(K)]
    for k, j in incid:
        band_chunks[k].append(j)
    # pad to a multiple of WGRP chunks; partition-major (128, NINC*128)
    n_wgrp = (n_inc + WGRP - 1) // WGRP
    if n_wgrp * WGRP > n_inc:
        pad = np.zeros((n_wgrp * WGRP - n_inc, 128, D), np.float32)
        wchunks = np.concatenate([wchunks, pad], axis=0)
    wflat = np.ascontiguousarray(wchunks.transpose(1, 0, 2).reshape(128, -1))
    wh, wl = _split_bf16(wflat)

    key = (B, C, T, F, K, D, tuple(starts.tolist()), tuple(ends.tolist()))
    if key not in _cache:
        _cache[key] = _build_program(Tc, n_chunks, band_chunks, n_inc)
    nc = _cache[key]

    biasT = np.ascontiguousarray(pre_b.T)
    ident = np.eye(128, dtype=NP_BF16)

    # host interleave: xi[b, t, j*128 + c*32 + w] = x[b, c, t, 32j + w]
    FQ = n_chunks * G
    xpad = np.zeros((B, C, T, FQ), np.float32)
    xpad[:, :, :, :F] = x
    xi = np.ascontiguousarray(
        xpad.reshape(B, C, T, n_chunks, G).transpose(0, 2, 3, 1, 4)
    ).reshape(B, T, n_chunks * C * G)
    xih, xil = _split_bf16(xi)

    in_maps = []
    for core in range(N_CORES):
        sl = slice(core * Tc, (core + 1) * Tc)
        in_maps.append(
            {
                "xh": np.ascontiguousarray(xih[:, sl]),
                "xl": np.ascontiguousarray(xil[:, sl]),
                "wh": wh,
                "wl": wl,
                "bias": biasT,
                "ident": ident,
            }
        )

    global _last_in_maps
    _last_in_maps = in_maps
    res = run_bass_kernel_spmd(nc, in_maps, list(range(N_CORES)))

    # per-core out: (K, D, B*Tc) -> full (B, D, T, K)
    arr = np.stack([res.results[i]["out"] for i in range(N_CORES)])
    arr = arr.reshape(N_CORES, K, D, B, Tc)
    out = np.transpose(arr, (3, 2, 0, 4, 1)).reshape(B, D, T, K)
    return np.ascontiguousarray(out)
